# revision 2
# baseline (speedup 1.0000x reference)
"""CORDIV stochastic-computing division kernel for Trainium2 (8 NeuronCores).

Recurrence per lane n (T sequential steps, lanes fully independent):
    sr = sr_init[:, n]                       # shift register, depth B
    for t in range(T):
        r  = rng_table[t % B]
        hq = sr[r]
        q[t, n] = dividend[t, n] if divisor[t, n] == 1 else hq
        sr = [q[t, n], sr[0], ..., sr[B-2]]

Unrolled, the shift register disappears (src_t = q[t-1-r_t] or an sr_init
row), and since every stream is bits {0,1} the per-step select is a bitwise
mux over host-packed bit-planes (32 lanes per uint32 word):
    q_t = (src_t & a_t) | b_t,   a = ~divisor, b = dividend & divisor
a and b have disjoint support per bit, so two DVE bitwise ops per step are
exact. Packing 32 lanes/word cuts HBM traffic 32x vs f32 streams:
per core ~0.9 MiB of loads + 0.5 MiB of stores -> a few-us DMA floor.

Steps are batched into groups whose source columns form a uniform-stride
access pattern, so the 16 steps need only ~14 DVE tensor_tensor ops on
[128, group*64] u32 tiles. The group schedule + SBUF column layout are
resolved on the host from rng_table (static DAG).

Sharding: lane dimension N split evenly across 8 cores (data parallel,
no communication). Loads ride the SP HWDGE queue, stores the ACT HWDGE
queue, chunked 2x for load/compute/store overlap.
"""

import numpy as np

import concourse.bass as bass
import concourse.mybir as mybir
from concourse.tile import TileContext
from concourse.bass_utils import run_bass_kernel_spmd

N_CORES = 8
P = 128  # SBUF partitions

_nc_cache: dict = {}
LAST_RESULTS = None  # test harness introspection
REPS = 1  # >1: wrap body in a HW loop (timing harness only; output unchanged)


def _plan(T, buf_dep, rng_table):
    """Host-side resolution of the recurrence into a static grouped DAG.

    Returns a tuple-of-tuples plan (hashable):
      groups: tuple of (kind, steps, srcs, src_base, src_stride,
                        in_off, dst_col)
        kind "s": srcs are sr_init row indices; the sr strips live in the
          input blob at strip offset in_off (then A at in_off+|g|, B at
          in_off+2|g|).
        kind "q": srcs are q step indices; src cols form an arithmetic
          progression (base src_base, stride src_stride) in the on-chip q
          tile; A at strip offset in_off, B at in_off+|g|.
      col: tuple mapping step t -> q-tile column index.
      n_strips: total input strips (each strip = one [P, W] u32 slab).
    """
    rng = [int(rng_table[t % buf_dep]) for t in range(T)]
    sched = []
    for t in range(T):
        j = t - 1 - rng[t]
        sched.append(("q", j) if j >= 0 else ("s", rng[t] - t))

    col = [-1] * T
    computed: set = set()
    remaining = list(range(T))
    ncol = 0
    raw_groups = []
    while remaining:
        sr_ready = [t for t in remaining if sched[t][0] == "s"]
        if sr_ready:
            g = sorted(sr_ready)
            kind = "s"
            stride = 1
        else:
            ready = [t for t in remaining if sched[t][1] in computed]
            assert ready, "dependency cycle in schedule"
            ready.sort(key=lambda t: (col[sched[t][1]], t))
            cand, seen = [], set()
            for t in ready:
                c = col[sched[t][1]]
                if c not in seen:
                    seen.add(c)
                    cand.append(t)
            if len(cand) < 2:
                g, stride = cand, 1
            else:
                best, best_stride = None, 1
                n = len(cand)
                for i in range(n):
                    for j2 in range(i + 1, n):
                        s = col[sched[cand[j2]][1]] - col[sched[cand[i]][1]]
                        run = [cand[i], cand[j2]]
                        last = col[sched[cand[j2]][1]]
                        for k in range(j2 + 1, n):
                            ck = col[sched[cand[k]][1]]
                            if ck == last + s:
                                run.append(cand[k])
                                last = ck
                        if best is None or len(run) > len(best):
                            best, best_stride = run, s
                g, stride = best, best_stride
            kind = "q"
        for t in g:
            col[t] = ncol
            ncol += 1
            computed.add(t)
            remaining.remove(t)
        raw_groups.append((kind, tuple(g), stride))
    assert ncol == T

    groups = []
    in_off = 0
    for kind, g, stride in raw_groups:
        if kind == "s":
            srcs = tuple(sched[t][1] for t in g)
            groups.append(("s", g, srcs, 0, 1, in_off, col[g[0]]))
            in_off += 3 * len(g)
        else:
            srcs = tuple(sched[t][1] for t in g)
            src_base = col[srcs[0]]
            groups.append(("q", g, srcs, src_base, stride, in_off, col[g[0]]))
            in_off += 2 * len(g)
    return tuple(groups), tuple(col), in_off


def _load_chunks(groups):
    """First group alone (smallest possible gate for the DVE chain), then
    pairs. Returns a list of lists of group indices."""
    chunks = [[0]]
    i = 1
    while i < len(groups):
        chunks.append(list(range(i, min(i + 2, len(groups)))))
        i += 2
    return chunks


def _store_spans(groups):
    """Three spans: the bulk in two ~even column splits, last group alone
    (small final store shortens the completion-latency tail)."""
    if len(groups) <= 2:
        return [list(range(len(groups)))]
    body = list(range(len(groups) - 1))
    cols = [len(groups[i][1]) for i in body]
    total = sum(cols)
    acc, cut = 0, 1
    for i in body:
        acc += cols[i]
        if acc >= total // 2:
            cut = i + 1
            break
    spans = [body[:cut]]
    if body[cut:]:
        spans.append(body[cut:])
    spans.append([len(groups) - 1])
    return spans


def _legalize_waits(nc):
    """Make the emitted BIR digestible by this walrus build.

    1. InstIncSwdgeSem (For_i loop skip/back-edge SWDGE sem adjustment)
       serializes with an empty ISA payload here ("ISA wrong length").
       Rewrite as NoOps carrying equivalent SyncUpdates.
    2. codegen accepts at most ONE sync wait per instruction. Extra waits
       are hoisted onto preceding same-engine NoOps.
    """
    n = 0
    mode_map = {"add": "sem-add-imm", "sub": "sem-sub-imm", "wr": "sem-wr-imm"}
    for blk in nc.m.functions[0].blocks:
        new_insts = []
        for inst in blk.instructions:
            if type(inst).__name__ == "InstIncSwdgeSem":
                if inst._mode == "add":
                    continue
                assert inst._mode == "sub", inst._mode
                for i, (val, name) in enumerate(
                    zip(inst._sem_values, inst._sem_names)
                ):
                    if val == 0:
                        continue
                    upd = mybir.SyncUpdate(
                        sync_type="semaphore",
                        id=inst._sem_id_base + i,
                        update_mode="sem-sub-imm",
                        update_value=val,
                        ant_name=name,
                    )
                    new_insts.append(
                        mybir.InstNoOp(
                            name=f"{inst.name}_swdgesem_{n}",
                            engine=inst.engine,
                            ins=[],
                            outs=[],
                            sync_info=mybir.SyncInfo(
                                on_wait=[], on_update=[upd]
                            ),
                        )
                    )
                    n += 1
            else:
                new_insts.append(inst)
        blk.instructions = new_insts
    for blk in nc.m.functions[0].blocks:
        new_insts = []
        for inst in blk.instructions:
            si = inst.sync_info
            waits = list(si.on_wait) if si is not None and si.on_wait is not None else []
            if len(waits) > 1 and inst.opcode != "ISA":
                for w in waits[:-1]:
                    nop = mybir.InstNoOp(
                        name=f"{inst.name}_waitnop_{n}",
                        engine=inst.engine,
                        ins=[],
                        outs=[],
                        sync_info=mybir.SyncInfo(on_wait=[w], on_update=[]),
                    )
                    new_insts.append(nop)
                    n += 1
                inst.sync_info = mybir.SyncInfo(
                    on_wait=[waits[-1]], on_update=list(si.on_update or [])
                )
            new_insts.append(inst)
        blk.instructions = new_insts
    return nc


EBYTES = 2  # on-chip element size: 2 (uint16) is the fast DVE path on HW
_EDT = {1: mybir.dt.uint8, 2: mybir.dt.uint16, 4: mybir.dt.uint32}
_NPDT = {1: np.uint8, 2: np.uint16, 4: np.uint32}


def _build(T, NS, plan, reps=1, legalize=True, ebytes=EBYTES):
    """Emit the per-core Bass/Tile module. NS = lanes per core."""
    groups, col, n_strips = plan
    W = NS // (8 * ebytes) // P  # elems per step per partition
    assert W * 8 * ebytes * P == NS, NS
    u32 = _EDT[ebytes]
    IN_W = n_strips * W
    OUT_W = T * W
    AND = mybir.AluOpType.bitwise_and
    OR = mybir.AluOpType.bitwise_or

    load_chunks = _load_chunks(groups)
    store_spans = _store_spans(groups)
    store_after = {span[-1]: span for span in store_spans}

    def strips_of(gi):
        kind, g, _, _, _, in_off, _ = groups[gi]
        return in_off, in_off + (3 if kind == "s" else 2) * len(g)

    nc = bass.Bass()
    inp = nc.dram_tensor("inp", [P, IN_W], u32, kind="ExternalInput")
    outp = nc.dram_tensor("quotient", [P, OUT_W], u32, kind="ExternalOutput")

    with TileContext(nc) as tc:
        with (
            tc.tile_pool(name="in", bufs=2) as pin,
            tc.tile_pool(name="q", bufs=2) as pq,
        ):

            def body():
                tin = pin.tile([P, IN_W], u32, tag="in")
                tq = pq.tile([P, OUT_W], u32, tag="q")
                tq3 = tq[:].rearrange("p (c w) -> p c w", w=W)
                tin3 = tin[:].rearrange("p (c w) -> p c w", w=W)

                # chunked loads alternating between the two HWDGE rings
                # (SP and ACT); compute on a chunk starts as soon as its
                # own load lands (subtile deps). The very first group's
                # AND operands (sr + a strips) ship alone so the DVE chain
                # is gated by the smallest possible load; its b strips ride
                # the other ring concurrently.
                k0, g0 = groups[0][0], groups[0][1]

                first_cut = (groups[0][5] + (2 if k0 == "s" else 1) * len(g0)) * W
                c1 = strips_of(0)[1] * W
                nc.sync.dma_start(tin[:, 0:first_cut], inp[:, 0:first_cut])
                nc.scalar.dma_start(tin[:, first_cut:c1], inp[:, first_cut:c1])
                lqueues = [nc.sync, nc.scalar]
                for ci, chunk in enumerate(load_chunks[1:]):
                    c0 = strips_of(chunk[0])[0] * W
                    c1 = strips_of(chunk[-1])[1] * W
                    lqueues[ci % 2].dma_start(tin[:, c0:c1], inp[:, c0:c1])

                squeues = [nc.scalar, nc.sync]
                nstore = 0
                for gi, (kind, g, srcs, src_base, src_stride, in_off, dst_col) in enumerate(
                    groups
                ):
                    gl = len(g)
                    if kind == "s":
                        src = tin3[:, in_off : in_off + gl, :]
                        a_off = in_off + gl
                    else:
                        if gl == 1:
                            src = tq3[:, src_base : src_base + 1, :]
                        else:
                            hi = src_base + src_stride * (gl - 1)
                            src = tq3[
                                :,
                                src_base : hi + (1 if src_stride > 0 else -1) : src_stride,
                                :,
                            ]
                        a_off = in_off
                    b_off = a_off + gl
                    dst = tq3[:, dst_col : dst_col + gl, :]
                    a_ap = tin3[:, a_off : a_off + gl, :]
                    b_ap = tin3[:, b_off : b_off + gl, :]
                    nc.vector.tensor_tensor(dst, src, a_ap, AND)
                    nc.vector.tensor_tensor(dst, dst, b_ap, OR)

                    span = store_after.get(gi)
                    if span is not None:
                        q_lo = groups[span[0]][6]
                        q_hi = groups[span[-1]][6] + len(groups[span[-1]][1])
                        squeues[nstore % 2].dma_start(
                            outp[:, q_lo * W : q_hi * W],
                            tq[:, q_lo * W : q_hi * W],
                        )
                        nstore += 1

            if reps == 1:
                body()
            else:
                with tc.For_i(0, reps, 1):
                    body()
    return _legalize_waits(nc) if legalize else nc


def _pack_bits(arr_u8, NC, ns_p):
    """[R, N] {0,1} u8 -> [R, NC, P, W] u32 bit-planes (32 lanes/word)."""
    R, N = arr_u8.shape
    x = arr_u8.reshape(R, NC, P, ns_p)
    x = np.packbits(x, axis=-1)  # [R, NC, P, ns_p//8] u8
    return np.ascontiguousarray(x).view(np.uint32)


def kernel(dividend, divisor, sr_init, rng_table):
    global LAST_RESULTS
    rng_host = np.asarray(rng_table).astype(np.int64)

    dividend = np.asarray(dividend)
    divisor = np.asarray(divisor)
    sr_np = np.asarray(sr_init)
    T, N = dividend.shape
    buf_dep = sr_np.shape[0]
    assert N % (N_CORES * P * 32) == 0, N
    NS = N // N_CORES
    ns_p = NS // P  # lanes per partition
    W = ns_p // 32  # u32 words per step per partition

    plan = _plan(T, buf_dep, rng_host)
    groups, col, n_strips = plan
    key = (T, NS, plan, REPS, EBYTES)
    nc = _nc_cache.get(key)
    if nc is None:
        nc = _build(T, NS, plan, reps=REPS)
        _nc_cache[key] = nc

    # bit-plane packing: a = ~divisor, b = dividend & divisor (disjoint),
    # q_t = (src & a_t) | b_t exactly on bits
    dvs_u8 = divisor.astype(np.uint8)
    dvd_u8 = dividend.astype(np.uint8)
    a_pack = _pack_bits(dvs_u8 ^ 1, N_CORES, ns_p)  # [T, NC, P, W]
    b_pack = _pack_bits(dvd_u8 & dvs_u8, N_CORES, ns_p)
    sr_pack = _pack_bits(sr_np.astype(np.uint8), N_CORES, ns_p)  # [B, NC, P, W]

    # assemble the input blob: per group [SR strips][A strips][B strips]
    in_maps = []
    for c in range(N_CORES):
        strips = []
        for kind, g, srcs, _, _, _, _ in groups:
            if kind == "s":
                for r in srcs:
                    strips.append(sr_pack[r, c])
            for t in g:
                strips.append(a_pack[t, c])
            for t in g:
                strips.append(b_pack[t, c])
        blob = np.stack(strips, axis=1)  # [P, n_strips, W] u32
        blob = np.ascontiguousarray(blob).reshape(P, n_strips * W)
        in_maps.append({"inp": blob.view(_NPDT[EBYTES])})

    res = run_bass_kernel_spmd(nc, in_maps, core_ids=list(range(N_CORES)))
    LAST_RESULTS = res

    # gather + unpack: out cols are in plan order; invert col[] per step
    out_all = np.stack(
        [
            np.ascontiguousarray(m["quotient"]).view(np.uint32).reshape(P, T, W)
            for m in res.results
        ]
    )  # [NC, P, T, W] u32
    qsteps = out_all[:, :, np.asarray(col), :]  # [NC, P, T, W] step-ordered
    qb = np.ascontiguousarray(qsteps.transpose(2, 0, 1, 3)).view(np.uint8)
    bits = np.unpackbits(qb, axis=-1)  # [T, NC, P, ns_p]
    return bits.reshape(T, N).astype(np.float32)
